# revision 48
# baseline (speedup 1.0000x reference)
"""Multi-head self-attention kernel for 8 Trainium2 NeuronCores.

Problem: B=2, S=2048, D=1024, H=16 heads, head_dim=64 (fp32 in/out).

Sharding: tensor-parallel over heads. Core c owns heads {2c, 2c+1}, i.e.
output-feature range [c*128, (c+1)*128) of the Q/K/V projections and the
matching 128 contraction rows of the output projection. Each core computes a
full-shape partial of the output in fp16; the host sums the 8 partials in
fp32 and adds bo.

All on-device activations and weights are fp16 (PE streams 1 col/cycle
regardless of 16/32-bit dtype, but fp16 halves DMA + SBUF traffic and the
HW-measured DMA rate is the projection-phase bottleneck). PSUM accumulation
stays fp32 (TRN2 matmul requires fp32 PSUM output).

Per-core device program:
  1. QT/KT/VT [128, 4096] fp16 = W_shard @ x.T  (x.T pre-transposed on host)
  2. V' [k, kc, 65] fp16 per (batch, head): V plus a ones column, built from
     VT by DVE 32x32 stream transposes. The ones column makes the softmax
     denominator fall out of the ctx matmul for free.
  3. Per (batch, head), per grain (k-chunk 128 x q-half 1024), software-
     pipelined one grain ahead so the strict PE FIFO never stalls on the
     exp handoff:
       scoresT [128, 1024] fp32 = KT_chunk.T @ QT   (2 matmuls, 2 PSUM banks)
       PT fp16 = exp(0.125 * scoresT)               (one ScalarE instr)
       ctx'T [65, q] += V'_chunk.T @ PT             (2 accumulating matmuls,
                                                     emitted one grain late)
     Rows 0..63 of ctx'T are the unnormalized context, row 64 the sums.
  4. recip(sums) on DVE -> K=1 PE outer product broadcasts to 64 partitions
     in a free scores-pool PSUM slot -> ScalarE copies to SBUF -> one DVE
     multiply writes scaled ctxT (fp16) into a persistent [128, 4096] buffer.
  5. out_partial [t=128, 1024] fp16 = ctxT_chunk.T @ WoT_shard per t-chunk.
"""

import functools
import itertools
import os
import sys

import numpy as np

for _p in ("/opt/trn_rl_repo", os.path.expanduser("~/.axon_site/_ro/trn_rl_repo")):
    if os.path.isdir(_p) and _p not in sys.path:
        sys.path.insert(0, _p)

import concourse.bass as bass
import concourse.tile as tile
from concourse import bacc
from concourse import mybir
from concourse.bass_utils import run_bass_kernel_spmd

F32 = mybir.dt.float32
F16 = mybir.dt.float16
F8 = mybir.dt.float8e4
AF = mybir.ActivationFunctionType

# fp8 DoubleRow ctx matmuls: two 128-token k-chunks per matmul (K_eff=256),
# halving the ctx stream cycles. PT and V' are fp8e4m3; exp carries bias=-1
# so PT = exp(logit - 1) stays within fp8 range (the uniform e^-1 cancels in
# the softmax normalization). Flip to False to fall back to fp16 ctx.
USE_FP8_CTX = False

P = 128          # partitions / feature slice per core
B = 2            # batch
S = 2048         # sequence length
D = 1024         # embed dim
T = B * S        # total tokens
HD = 64          # head dim
KO = D // P      # contraction subtiles for the projections
NT = 8           # t-tiles for the projections
TW = 512         # projection t-tile width / matmul free dim
NKC = S // P     # 128-wide k-chunks per (batch, head)
NQS = S // TW    # 512-wide q-slices per (batch, head)
HW2 = S // 2     # 1024-wide q-half (exp grain)
N_CORES = 8
SCALE = 1.0 / np.sqrt(np.float32(HD))  # 0.125


def _build_nc(n_reps: int = 1, phases: str = "full", dyn_reps: bool = False):
    nc = bacc.Bacc(target_bir_lowering=False, debug=False, num_devices=N_CORES)

    if dyn_reps:
        reps = nc.declare_dram_parameter("reps", [1, 1], mybir.dt.int32, isOutput=False)
    # xt4[tt, ki, ko, t] = x[tt*TW + t, ko*P + ki]; per-partition-contiguous DMA
    xt4 = nc.declare_dram_parameter("xt4", [NT, P, KO, TW], F16, isOutput=False)
    wqT = nc.declare_dram_parameter("wqT", [P, KO, P], F16, isOutput=False)
    wkT = nc.declare_dram_parameter("wkT", [P, KO, P], F16, isOutput=False)
    wvT = nc.declare_dram_parameter("wvT", [P, KO, P], F16, isOutput=False)
    woT = nc.declare_dram_parameter("woT", [P, D], F16, isOutput=False)
    bq = nc.declare_dram_parameter("bq", [P, 1], F32, isOutput=False)
    bk = nc.declare_dram_parameter("bk", [P, 1], F32, isOutput=False)
    bv = nc.declare_dram_parameter("bv", [P, 1], F32, isOutput=False)
    out = nc.declare_dram_parameter("out", [T, D], F16, isOutput=True)

    with tile.TileContext(nc) as tc:
        from contextlib import ExitStack, nullcontext

        with ExitStack() as ctx:
            singles = ctx.enter_context(tc.tile_pool(name="singles", bufs=1))
            qkv = ctx.enter_context(tc.tile_pool(name="qkv", bufs=1))
            xpool = ctx.enter_context(tc.tile_pool(name="xpool", bufs=4))
            ptpool = ctx.enter_context(tc.tile_pool(name="ptpool", bufs=11))
            vpool = ctx.enter_context(tc.tile_pool(name="vpool", bufs=2))
            rpool = ctx.enter_context(tc.tile_pool(name="rpool", bufs=2))
            opool = ctx.enter_context(tc.tile_pool(name="opool", bufs=5))
            sppool = ctx.enter_context(
                tc.tile_pool(name="sppool", bufs=2, space="PSUM")
            )
            pbank = ctx.enter_context(tc.tile_pool(name="pbank", bufs=4, space="PSUM"))
            pools = (singles, qkv, xpool, ptpool, vpool, rpool, opool, sppool,
                     pbank, xt4, wqT, wkT, wvT, woT, bq, bk, bv, out)

            if dyn_reps:
                reps_sb = singles.tile([1, 1], mybir.dt.int32, name="reps_sb")
                nc.sync.dma_start(out=reps_sb[:], in_=reps[:])
                reps_val = nc.values_load(reps_sb[:], min_val=0, max_val=1 << 20)
                rep_loop = tc.For_i(0, reps_val, 1)
            elif n_reps > 1:
                rep_loop = tc.For_i(0, n_reps, 1)
            else:
                rep_loop = nullcontext()
            with rep_loop:
                _kernel_body(nc, tc, pools, phases)

    nc.finalize()
    return nc


def _emit_recips(nc, rpool, ctx_ps, bb, h):
    # DVE-only: reciprocal of the softmax sums, then a fp16 copy for the PE
    # broadcast matmul. Emitted eagerly at pair end.
    rs = rpool.tile([1, S], F32, tag="rs", name=f"rs_{bb}_{h}")
    for qs in range(NQS):
        nc.vector.reciprocal(
            rs[:, qs * TW : (qs + 1) * TW], ctx_ps[qs][HD : HD + 1, :]
        )
    rs_h = rpool.tile([1, S], F16, tag="rs_h", name=f"rs_h_{bb}_{h}")
    nc.vector.tensor_copy(rs_h[:], rs[:])
    return rs_h


def _emit_division(nc, rpool, ctx_ps, CX, pb, base, bb, h, rs_h):
    # GpSimd broadcasts the reciprocal row to 64 partitions (SBUF->SBUF,
    # GpSimd is otherwise idle) -- no PE outer product, no PSUM tile, so the
    # division never contends with the next pair's scores double-buffer.
    rb = rpool.tile([HD, S], F16, tag="rb", name=f"rb_{bb}_{h}")
    nc.gpsimd.partition_broadcast(rb[:, :], rs_h[:, :])
    for qs in range(NQS):
        nc.vector.tensor_mul(
            out=CX[pb : pb + HD, base + qs * TW : base + (qs + 1) * TW],
            in0=ctx_ps[qs][0:HD, :],
            in1=rb[:, qs * TW : (qs + 1) * TW],
        )


F32R = mybir.dt.float32r


def _microbench(nc, pools, phases):
    # Engine-rate microbenches (timing only, out never written).
    (singles, qkv, xpool, ptpool, vpool, rpool, opool, sppool, pbank,
     *_rest) = pools
    if phases.startswith("actbench"):
        sp = sppool.tile([P, HW2], F32, tag="sp", name="ab_src")
        nc.vector.memset(sp[:], 0.25)
        odt = F32 if phases == "actbench32" else F16
        for i in range(256):
            pt = ptpool.tile([P, HW2], odt, tag="pt", name=f"ab_{i}")
            nc.scalar.activation(pt[:], sp[:], AF.Exp, scale=float(SCALE))
        return
    # mmbench16 / mmbenchr: 512 score-like matmuls (K=64, N=512)
    lhs32 = singles.tile([HD, P], F32, tag="mb_lhs")
    rhs32 = singles.tile([HD, HW2], F32, tag="mb_rhs")
    nc.vector.memset(lhs32[:], 0.125)
    nc.vector.memset(rhs32[:], 0.125)
    if phases == "mmbench16":
        lhs = singles.tile([HD, P], F16, tag="mb_lhs16")
        rhs = singles.tile([HD, HW2], F16, tag="mb_rhs16")
        nc.vector.tensor_copy(lhs[:], lhs32[:])
        nc.vector.tensor_copy(rhs[:], rhs32[:])
    else:
        lhs = lhs32.bitcast(F32R)
        rhs = rhs32.bitcast(F32R)
    if phases == "mmbench128":
        # 512 N=128 matmuls: per-MM slope vs mmbench16 isolates PE clock
        for i in range(128):
            sp = sppool.tile([P, TW], F32, tag="sp", name=f"mb_{i}")
            for qs in range(4):
                nc.tensor.matmul(
                    sp[:, qs * 128 : (qs + 1) * 128],
                    lhs[:],
                    rhs[:, qs * 128 : (qs + 1) * 128],
                    start=True,
                    stop=True,
                )
        return
    if phases == "mmbenchst":
        # K=128 rotating stationary, all start=True: isolates the
        # start-clear cost from the K=64 effect
        wts = []
        for i in range(8):
            w32 = singles.tile([P, P], F32, tag=f"mb_w{i}")
            nc.vector.memset(w32[:], 0.03125)
            w16 = singles.tile([P, P], F16, tag=f"mb_w16{i}")
            nc.vector.tensor_copy(w16[:], w32[:])
            wts.append(w16)
        big32 = singles.tile([P, TW], F32, tag="mb_big")
        nc.vector.memset(big32[:], 0.0625)
        big = singles.tile([P, TW], F16, tag="mb_big16")
        nc.vector.tensor_copy(big[:], big32[:])
        for i in range(512):
            sp = sppool.tile([P, TW], F32, tag="sp", name=f"mbs_{i}")
            nc.tensor.matmul(
                sp[:], wts[i % 8][:], big[:], start=True, stop=True
            )
        return
    if phases == "mmbenchacc":
        # proj-like: 64 groups x 8 accumulating K=128 N=512 matmuls,
        # stationary rotates through 8 tiles
        wts = []
        for i in range(8):
            w32 = singles.tile([P, P], F32, tag=f"mb_w{i}")
            nc.vector.memset(w32[:], 0.03125)
            w16 = singles.tile([P, P], F16, tag=f"mb_w16{i}")
            nc.vector.tensor_copy(w16[:], w32[:])
            wts.append(w16)
        big32 = singles.tile([P, TW], F32, tag="mb_big")
        nc.vector.memset(big32[:], 0.0625)
        big = singles.tile([P, TW], F16, tag="mb_big16")
        nc.vector.tensor_copy(big[:], big32[:])
        for i in range(64):
            sp = sppool.tile([P, TW], F32, tag="sp", name=f"mba_{i}")
            for ko in range(8):
                nc.tensor.matmul(
                    sp[:],
                    wts[ko][:],
                    big[:],
                    start=(ko == 0),
                    stop=(ko == 7),
                )
        return
    for i in range(256):
        sp = sppool.tile([P, HW2], F32, tag="sp", name=f"mb_{i}")
        for qs in range(2):
            nc.tensor.matmul(
                sp[:, qs * TW : (qs + 1) * TW],
                lhs[:],
                rhs[:, qs * TW : (qs + 1) * TW],
                start=True,
                stop=True,
            )


def _kernel_body(nc, tc, pools, phases="full"):
    (singles, qkv, xpool, ptpool, vpool, rpool, opool, sppool, pbank,
     xt4, wqT, wkT, wvT, woT, bq, bk, bv, out) = pools

    if phases.startswith(("actbench", "mmbench")):
        _microbench(nc, pools, phases)
        return

    # ---- weights / biases to SBUF ----
    w_sbs = []
    for name, wT in (("wq", wqT), ("wk", wkT), ("wv", wvT)):
        w_sb = singles.tile([P, KO, P], F16, tag=f"{name}_sb")
        nc.sync.dma_start(out=w_sb[:], in_=wT[:])
        w_sbs.append(w_sb)
    wo_sb = singles.tile([P, D], F16, tag="wo_sb")
    nc.sync.dma_start(out=wo_sb[:], in_=woT[:])
    b_sbs = []
    for name, bdram in (("bq", bq), ("bk", bk), ("bv", bv)):
        b_sb = singles.tile([P, 1], F32, tag=f"{name}_sb")
        nc.sync.dma_start(out=b_sb[:], in_=bdram[:])
        b_sbs.append(b_sb)

    # ones row for PE-broadcast of softmax reciprocals (K=1 matmul)
    negone = singles.tile([P, 1], F32, tag="negone")
    nc.vector.memset(negone[:], -2.5)

    # ---- persistent activations ----
    QT = qkv.tile([P, T], F16, tag="QT")
    # K is kept zero-padded to K=128 per head (ZK0 = [K_h0; 0], ZK1 =
    # [0; K_h1]) so every scores matmul has a full 128-row stationary:
    # K=64 stationaries disqualify fast-weight-load and cost ~190ns/MM extra
    # (HW-measured). The zero half multiplies the other head's Q rows.
    ZK0 = qkv.tile([P, T], F16, tag="ZK0")
    ZK1 = qkv.tile([P, T], F16, tag="ZK1")
    VT = qkv.tile([P, T], F16, tag="VT")
    CX = qkv.tile([P, T], F16, tag="CX")  # scaled ctxT, both heads
    nc.gpsimd.memset(ZK0[HD:P, :], 0.0)
    nc.gpsimd.memset(ZK1[0:HD, :], 0.0)

    # ---- projections: QT/KT/VT[f, t] = sum_d W[d, f] * xT[d, t] ----

    xts = {}

    def proj_tile(tt, parts="QKV"):
        if tt in xts:
            xt = xts[tt]
        else:
            xt = xpool.tile([P, KO, TW], F16, tag="xt", name=f"xt_{tt}")
            # four dma_starts -> four HWDGE queues in parallel (8-way split
            # measured same-window neutral-to-worse)
            nq = 4
            w = KO // nq
            for q in range(nq):
                nc.sync.dma_start(
                    out=xt[:, w * q : w * (q + 1)],
                    in_=xt4[:][tt, :, w * q : w * (q + 1)],
                )
            xts[tt] = xt
        for which, w_sb, b_sb, dst in zip(
            "QKV", w_sbs, b_sbs, (QT, None, VT), strict=True
        ):
            if which not in parts:
                continue
            ps = sppool.tile([P, TW], F32, tag="sp")
            for ko in range(KO):
                nc.tensor.matmul(
                    ps[:],
                    w_sb[:, ko],
                    xt[:, ko],
                    start=(ko == 0),
                    stop=(ko == KO - 1),
                )
            sl = slice(tt * TW, (tt + 1) * TW)
            if dst is None:  # K projection: split into the zero-padded bufs
                nc.vector.tensor_scalar_add(
                    ZK0[0:HD, sl], ps[0:HD, :], b_sb[0:HD]
                )
                nc.vector.tensor_scalar_add(
                    ZK1[HD:P, sl], ps[HD:P, :], b_sb[HD:P]
                )
            else:
                nc.vector.tensor_scalar_add(dst[:, sl], ps[:], b_sb[:])
            yield

    # batch-0 Q and K projections first; the V projections and the V' build
    # are deferred into pair(0,0)'s early grains (V' is first consumed at the
    # first ctx matmul, `lag` grains in)
    for tt in range(NT // 2):
        for _ in proj_tile(tt, "QK"):
            pass

    # ---- V' build for one (batch, head): V plus a ones column, via DVE
    #      32x32 stream transposes of VT (fp16 end to end) ----
    def vbuild_into(vp, bb: int, h: int):
        base = bb * S
        pb = h * HD
        nc.vector.memset(vp[:, :, HD], 1.0)
        for a in range(2):
            src = VT[pb + 32 * a : pb + 32 * (a + 1), base : base + S]
            src = src.rearrange("p (kc r) -> p kc r", r=P)
            for b2 in range(4):
                nc.vector.transpose(
                    vp[32 * b2 : 32 * (b2 + 1), :, 32 * a : 32 * (a + 1)],
                    src[:, :, 32 * b2 : 32 * (b2 + 1)],
                )
        if not USE_FP8_CTX:
            return vp
        # fp8 copy, inner dim padded to 80 so the DoubleRow Ko stride is
        # 16-byte aligned
        vp8 = vpool.tile([P, NKC, 80], F8, tag="vp8", name=f"vp8_{bb}_{h}")
        nc.vector.tensor_copy(vp8[:, :, 0 : HD + 1], vp[:])
        return vp8

    def vbuild(bb: int, h: int):
        vp = vpool.tile([P, NKC, HD + 1], F16, tag="vp", name=f"vp_{bb}_{h}")
        return vbuild_into(vp, bb, h)

    # ---- attention for one (batch, head); generator yields per grain
    #      (k-chunk x q-half), ctx matmuls lag one grain behind scores/exp
    #      so the PE FIFO never stalls head-of-queue on the exp handoff ----
    def pair_attn(bb: int, h: int, vp, lag: int = 1, drain_early: bool = False):
        base = bb * S       # token offset of this batch
        pb = h * HD         # partition offset of this head in QT/KT/VT
        yield

        ctx_ps = [
            pbank.tile([HD + 1, TW], F32, tag="pb", name=f"ctx_ps_{bb}_{h}_{i}")
            for i in range(NQS)
        ]

        ZK = ZK1 if h else ZK0

        def emit_scores(kc, half, sp):
            kt_chunk = ZK[:, base + kc * P : base + (kc + 1) * P]
            q0 = half * HW2
            for qs in range(2):
                nc.tensor.matmul(
                    sp[:, qs * TW : (qs + 1) * TW],
                    kt_chunk,
                    QT[
                        :,
                        base + q0 + qs * TW : base + q0 + (qs + 1) * TW,
                    ],
                    start=True,
                    stop=True,
                )

        from collections import deque

        if USE_FP8_CTX:
            NPR = NKC // 2

            def emit_se8(c, j, half, pp):
                sp = sppool.tile(
                    [P, HW2], F32, tag="sp", name=f"sp_{bb}_{h}_{c}_{j}_{half}"
                )
                emit_scores(2 * c + j, half, sp)
                nc.scalar.activation(
                    pp[:, j], sp[:], AF.Exp, scale=float(SCALE), bias=negone[:]
                )

            def emit_ctx8(c, half, pp):
                vpair = vp[:, 2 * c : 2 * c + 2, 0 : HD + 1]
                for qs in range(2):
                    nc.tensor.matmul(
                        ctx_ps[half * 2 + qs][:],
                        vpair,
                        pp[:, :, qs * TW : (qs + 1) * TW],
                        start=(c == 0),
                        stop=(c == NPR - 1),
                        perf_mode=mybir.MatmulPerfMode.DoubleRow,
                    )

            pend = deque()
            pps = {}
            for c in range(NPR):
                for j in range(2):
                    for half in range(2):
                        if j == 0:
                            pps[half] = ptpool.tile(
                                [P, 2, HW2], F8, tag="pt",
                                name=f"pp_{bb}_{h}_{c}_{half}",
                            )
                        emit_se8(c, j, half, pps[half])
                        if j == 1:
                            pend.append((c, half, pps[half]))
                        if len(pend) >= 2:
                            emit_ctx8(*pend.popleft())
                        yield
            while pend:
                emit_ctx8(*pend.popleft())
        else:

            def emit_se(kc, half):
                sp = sppool.tile(
                    [P, HW2], F32, tag="sp", name=f"sp_{bb}_{h}_{kc}_{half}"
                )
                emit_scores(kc, half, sp)
                pt = ptpool.tile(
                    [P, HW2], F16, tag="pt", name=f"pt_{bb}_{h}_{kc}_{half}"
                )
                nc.scalar.activation(pt[:], sp[:], AF.Exp, scale=float(SCALE))
                return pt

            def emit_ctx(kc, half, pt):
                vchunk = vp[:, kc]
                for qs in range(2):
                    nc.tensor.matmul(
                        ctx_ps[half * 2 + qs][:],
                        vchunk,
                        pt[:, qs * TW : (qs + 1) * TW],
                        start=(kc == 0),
                        stop=(kc == NKC - 1),
                    )

            pending = deque()
            gi = 0
            for kc in range(NKC):
                for half in range(2):
                    pt = emit_se(kc, half)
                    gi += 1
                    # drain_early: work the backlog down mid-pair (2 ctx/grain
                    # once past grain 20) so the final division isn't gated by
                    # a lag-deep ctx burst at the very end
                    limit = 2 if (drain_early and gi > 20) else lag
                    pops = 0
                    while len(pending) >= limit and pops < 2:
                        emit_ctx(*pending.popleft())
                        pops += 1
                    pending.append((kc, half, pt))
                    yield
            while pending:
                emit_ctx(*pending.popleft())

        rs_h = _emit_recips(nc, rpool, ctx_ps, bb, h)

        def division():
            _emit_division(nc, rpool, ctx_ps, CX, pb, base, bb, h, rs_h)

        yield division

    # ---- output projection for one batch (generator) ----
    def outproj(bb: int, tail: bool = False):
        for tci in range(S // P):
            tg = bb * (S // P) + tci
            ot = opool.tile([P, D], F16, tag="ot")
            for half in range(2):
                ps = sppool.tile([P, TW], F32, tag="sp")
                nc.tensor.matmul(
                    ps[:],
                    CX[:, tg * P : (tg + 1) * P],
                    wo_sb[:, half * TW : (half + 1) * TW],
                    start=True,
                    stop=True,
                )
                # DVE evacuation while interleaved with attention (ScalarE is
                # saturated by exp there); at the tail ScalarE is idle, so
                # split the halves across both engines
                if tail and half == 0:
                    nc.scalar.copy(ot[:, half * TW : (half + 1) * TW], ps[:])
                else:
                    nc.vector.tensor_copy(
                        ot[:, half * TW : (half + 1) * TW], ps[:]
                    )
                nc.sync.dma_start(
                    out=out[:][
                        tg * P : (tg + 1) * P, half * TW : (half + 1) * TW
                    ],
                    in_=ot[:, half * TW : (half + 1) * TW],
                )
            yield

    if phases == "proj":
        for tt in range(NT // 2, NT):
            for _ in proj_tile(tt):
                pass
        for i, t_ in enumerate((QT, ZK0, VT)):
            for j in range(4):
                nc.sync.dma_start(
                    out=out[:][(4 * i + j) * P : (4 * i + j + 1) * P, :],
                    in_=t_[:, j * D : (j + 1) * D],
                )
        return

    def run_pair(gen, prev_div=None, interleave=None, plan=None):
        # Drive a pair generator. Yields are: one pre-yield, one per grain,
        # then the pair's deferred division closure. The previous pair's
        # division is emitted right after this pair's first grain; `plan`
        # maps grain number -> how many interleave steps to emit there.
        division = None
        n = 0
        for item in gen:
            if callable(item):
                division = item
                continue
            n += 1
            if prev_div is not None and n == 1:
                prev_div()
                prev_div = None
            if interleave is not None and plan:
                for _ in range(plan.get(n, 0)):
                    next(interleave, None)
        if prev_div is not None:
            prev_div()
        return division

    def spread(grains, count, start=1, step=2):
        # plan placing `count` steps at grains start, start+step, ...
        return {start + i * step: 1 for i in range(count)}

    if phases == "oldsched":
        # previous schedule (serial b0 V-proj + vbuild before pair(0,0)),
        # kept for same-window A/B against the deferred-V flow
        for tt in range(NT // 2):
            for _ in proj_tile(tt, "V"):
                pass
        proj_steps = itertools.chain(
            *[proj_tile(tt) for tt in range(NT // 2, NT)]
        )
        vp0 = vbuild(0, 0)
        d00 = run_pair(
            pair_attn(0, 0, vp0), None, proj_steps, plan=spread(32, 12, 2, 3)
        )
        for _ in proj_steps:
            pass
        vp_ = vbuild(0, 1)
        d01 = run_pair(pair_attn(0, 1, vp_), prev_div=d00)
        opg = outproj(0)
        vp_ = vbuild(1, 0)
        d10 = run_pair(
            pair_attn(1, 0, vp_), prev_div=d01, interleave=opg,
            plan=spread(32, 14, 5, 2),
        )
        vp_ = vbuild(1, 1)
        d11 = run_pair(pair_attn(1, 1, vp_), prev_div=d10)
        d11()
        for _ in opg:
            pass
        for _ in outproj(1, tail=True):
            pass
        return

    if phases in ("attn1", "attn1i"):
        # diagnostic: one attention pair, with vs without proj interleave
        # (same total emitted work either way)
        for tt in range(NT // 2):
            for _ in proj_tile(tt, "V"):
                pass
        if phases == "attn1":
            for tt in range(NT // 2, NT):
                for _ in proj_tile(tt):
                    pass
        vp0 = vbuild(0, 0)
        if phases == "attn1":
            d = run_pair(pair_attn(0, 0, vp0))
        else:
            steps = itertools.chain(
                *[proj_tile(tt) for tt in range(NT // 2, NT)]
            )
            d = run_pair(
                pair_attn(0, 0, vp0), None, steps, plan=spread(32, 12, 2, 3)
            )
            for _ in steps:
                pass
        d()
        nc.sync.dma_start(out=out[:][0:P, :], in_=CX[:, 0:D])
        return

    # pair(0,0) starts right after the batch-0 Q/K projections. Its early
    # grains (every grain, 1..5) emit the deferred batch-0 V projections and
    # the V' build -- all done by grain 5, safely before the first ctx
    # matmul at grain lag+1 = 9. The batch-1 projections are then spread
    # over the rest of pair(0,0) and pair(0,1); outproj(0) over pairs (1,0)
    # and (1,1). Each pair's division is deferred into the next pair's first
    # grain (prev_div) so the recip->broadcast->mul chain never stalls the
    # PE FIFO at a pair boundary.
    vp00 = vpool.tile([P, NKC, HD + 1], F16, tag="vp", name="vp_0_0")

    def late_steps():
        for tt in range(NT // 2):
            for _ in proj_tile(tt, "V"):
                yield
        vbuild_into(vp00, 0, 0)
        yield
        for tt in range(NT // 2, NT):
            for _ in proj_tile(tt):
                yield

    def one_shot(fn_):
        fn_()
        yield

    steps = late_steps()  # 4 V-projs + 1 vbuild + 12 batch-1 projs
    plan00 = {n: 1 for n in range(1, 6)}
    plan00.update(spread(32, 6, 7, 4))  # grains 7,11,...,27
    div00 = run_pair(pair_attn(0, 0, vp00, lag=8, drain_early=phases != "de1"), None, steps, plan=plan00)

    # later pairs: V' build lands at grain 2 (after the previous division's
    # DVE chain releases the ctx banks), first ctx at grain lag+1 = 9
    vp01 = vpool.tile([P, NKC, HD + 1], F16, tag="vp", name="vp_0_1")
    steps01 = itertools.chain(
        one_shot(lambda: vbuild_into(vp01, 0, 1)), steps
    )
    div01 = run_pair(
        pair_attn(0, 1, vp01, lag=8, drain_early=phases != "de1"), prev_div=div00, interleave=steps01,
        plan={2: 1, **spread(32, 6, 4, 4)},
    )
    for _ in steps01:
        pass
    if phases == "attn2":
        div01()
        nc.sync.dma_start(out=out[:][0:P, :], in_=CX[:, 0:D])
        return
    op0 = outproj(0)
    vp10 = vpool.tile([P, NKC, HD + 1], F16, tag="vp", name="vp_1_0")
    steps10 = itertools.chain(
        one_shot(lambda: vbuild_into(vp10, 1, 0)), op0
    )
    div10 = run_pair(
        pair_attn(1, 0, vp10, lag=8, drain_early=phases != "de1"), prev_div=div01, interleave=steps10,
        plan={2: 1, **spread(32, 14, 4, 2)},
    )
    vp11 = vpool.tile([P, NKC, HD + 1], F16, tag="vp", name="vp_1_1")
    steps11 = itertools.chain(
        one_shot(lambda: vbuild_into(vp11, 1, 1)), steps10
    )
    div11 = run_pair(
        pair_attn(1, 1, vp11, lag=8, drain_early=True), prev_div=div10,
        interleave=steps11, plan={2: 1, 4: 1, 6: 1},
    )
    div11()
    for _ in steps11:
        pass
    for _ in outproj(1, tail=True):
        pass


@functools.lru_cache(maxsize=8)
def _get_nc(n_reps: int = 1, phases: str = "full", dyn_reps: bool = False):
    return _build_nc(n_reps, phases, dyn_reps)


def _shard_inputs(x, Wq, bq, Wk, bk, Wv, bv, Wo, bo):
    x = np.asarray(x, dtype=np.float32)
    # xt4[tt, ki, ko, t] = x[tt*TW + t, ko*P + ki]
    xt4 = np.ascontiguousarray(
        x.reshape(NT, TW, KO, P).transpose(0, 3, 2, 1)
    ).astype(np.float16)
    Wq = np.asarray(Wq, dtype=np.float32)
    Wk = np.asarray(Wk, dtype=np.float32)
    Wv = np.asarray(Wv, dtype=np.float32)
    Wo = np.asarray(Wo, dtype=np.float32)
    bq = np.asarray(bq, dtype=np.float32)
    bk = np.asarray(bk, dtype=np.float32)
    bv = np.asarray(bv, dtype=np.float32)

    def wtile(W, sl):
        # [ki, ko, f] = W[c*P + f, ko*P + ki]
        return np.ascontiguousarray(
            W[sl, :].reshape(P, KO, P).transpose(2, 1, 0)
        ).astype(np.float16)

    in_maps = []
    for c in range(N_CORES):
        sl = slice(c * P, (c + 1) * P)
        in_maps.append(
            {
                "xt4": xt4,
                "wqT": wtile(Wq, sl),
                "wkT": wtile(Wk, sl),
                "wvT": wtile(Wv, sl),
                "woT": np.ascontiguousarray(Wo[:, sl].T).astype(np.float16),
                "bq": np.ascontiguousarray(bq[sl].reshape(P, 1)),
                "bk": np.ascontiguousarray(bk[sl].reshape(P, 1)),
                "bv": np.ascontiguousarray(bv[sl].reshape(P, 1)),
            }
        )
    return in_maps


def kernel(x, Wq, bq, Wk, bk, Wv, bv, Wo, bo, **run_kwargs):
    nc = _get_nc()
    in_maps = _shard_inputs(x, Wq, bq, Wk, bk, Wv, bv, Wo, bo)
    last_exc = None
    for _attempt in range(3):
        try:
            res = run_bass_kernel_spmd(
                nc, in_maps, core_ids=list(range(N_CORES)), **run_kwargs
            )
            break
        except Exception as exc:  # transient device errors: retry
            last_exc = exc
            import time as _time

            _time.sleep(3.0)
            # a wedged PJRT client never recovers in-process; force a fresh
            # backend connection so the retry sees recovered devices
            try:
                import jax as _jax

                _jax.clear_caches()
                _jax.extend.backend.clear_backends()
            except Exception:
                pass
    else:
        raise last_exc
    partials = [r["out"] for r in res.results]
    acc = np.add.reduce([np.asarray(p, dtype=np.float32) for p in partials], axis=0)
    acc = acc + np.asarray(bo, dtype=np.float32)[None, :]
    if run_kwargs:
        kernel.last_results = res
    return acc.reshape(B, S, D).astype(np.float32)


# revision 51
# speedup vs baseline: 1.0819x; 1.0819x over previous
"""Multi-head self-attention kernel for 8 Trainium2 NeuronCores.

Problem: B=2, S=2048, D=1024, H=16 heads, head_dim=64 (fp32 in/out).

Sharding: tensor-parallel over heads. Core c owns heads {2c, 2c+1}, i.e.
output-feature range [c*128, (c+1)*128) of the Q/K/V projections and the
matching 128 contraction rows of the output projection. Each core computes a
full-shape partial of the output in fp16; the host sums the 8 partials in
fp32 and adds bo.

All on-device activations and weights are fp16 (PE streams 1 col/cycle
regardless of 16/32-bit dtype, but fp16 halves DMA + SBUF traffic and the
HW-measured DMA rate is the projection-phase bottleneck). PSUM accumulation
stays fp32 (TRN2 matmul requires fp32 PSUM output).

Per-core device program:
  1. QT/KT/VT [128, 4096] fp16 = W_shard @ x.T  (x.T pre-transposed on host)
  2. V' [k, kc, 65] fp16 per (batch, head): V plus a ones column, built from
     VT by DVE 32x32 stream transposes. The ones column makes the softmax
     denominator fall out of the ctx matmul for free.
  3. Per (batch, head), per grain (k-chunk 128 x q-half 1024), software-
     pipelined one grain ahead so the strict PE FIFO never stalls on the
     exp handoff:
       scoresT [128, 1024] fp32 = KT_chunk.T @ QT   (2 matmuls, 2 PSUM banks)
       PT fp16 = exp(0.125 * scoresT)               (one ScalarE instr)
       ctx'T [65, q] += V'_chunk.T @ PT             (2 accumulating matmuls,
                                                     emitted one grain late)
     Rows 0..63 of ctx'T are the unnormalized context, row 64 the sums.
  4. recip(sums) on DVE -> K=1 PE outer product broadcasts to 64 partitions
     in a free scores-pool PSUM slot -> ScalarE copies to SBUF -> one DVE
     multiply writes scaled ctxT (fp16) into a persistent [128, 4096] buffer.
  5. out_partial [t=128, 1024] fp16 = ctxT_chunk.T @ WoT_shard per t-chunk.
"""

import functools
import itertools
import os
import sys

import numpy as np

for _p in ("/opt/trn_rl_repo", os.path.expanduser("~/.axon_site/_ro/trn_rl_repo")):
    if os.path.isdir(_p) and _p not in sys.path:
        sys.path.insert(0, _p)

import concourse.bass as bass
import concourse.tile as tile
from concourse import bacc
from concourse import mybir
from concourse.bass_utils import run_bass_kernel_spmd

F32 = mybir.dt.float32
F16 = mybir.dt.float16
F8 = mybir.dt.float8e4
AF = mybir.ActivationFunctionType

# fp8 DoubleRow ctx matmuls: two 128-token k-chunks per matmul (K_eff=256),
# halving the ctx stream cycles. PT and V' are fp8e4m3; exp carries bias=-1
# so PT = exp(logit - 1) stays within fp8 range (the uniform e^-1 cancels in
# the softmax normalization). Flip to False to fall back to fp16 ctx.
USE_FP8_CTX = False

P = 128          # partitions / feature slice per core
B = 2            # batch
S = 2048         # sequence length
D = 1024         # embed dim
T = B * S        # total tokens
HD = 64          # head dim
KO = D // P      # contraction subtiles for the projections
NT = 8           # t-tiles for the projections
TW = 512         # projection t-tile width / matmul free dim
NKC = S // P     # 128-wide k-chunks per (batch, head)
NQS = S // TW    # 512-wide q-slices per (batch, head)
HW2 = S // 2     # 1024-wide q-half (exp grain)
N_CORES = 8
SCALE = 1.0 / np.sqrt(np.float32(HD))  # 0.125


def _build_nc(n_reps: int = 1, phases: str = "full", dyn_reps: bool = False):
    nc = bacc.Bacc(target_bir_lowering=False, debug=False, num_devices=N_CORES)

    if dyn_reps:
        reps = nc.declare_dram_parameter("reps", [1, 1], mybir.dt.int32, isOutput=False)
    # xt4[tt, ki, ko, t] = x[tt*TW + t, ko*P + ki]; per-partition-contiguous DMA
    xt4 = nc.declare_dram_parameter("xt4", [NT, P, KO, TW], F16, isOutput=False)
    wqT = nc.declare_dram_parameter("wqT", [P, KO, P], F16, isOutput=False)
    wkT = nc.declare_dram_parameter("wkT", [P, KO, P], F16, isOutput=False)
    wvT = nc.declare_dram_parameter("wvT", [P, KO, P], F16, isOutput=False)
    woT = nc.declare_dram_parameter("woT", [P, D], F16, isOutput=False)
    bq = nc.declare_dram_parameter("bq", [P, 1], F32, isOutput=False)
    bk = nc.declare_dram_parameter("bk", [P, 1], F32, isOutput=False)
    bv = nc.declare_dram_parameter("bv", [P, 1], F32, isOutput=False)
    out = nc.declare_dram_parameter("out", [T, D], F16, isOutput=True)

    with tile.TileContext(nc) as tc:
        from contextlib import ExitStack, nullcontext

        with ExitStack() as ctx:
            singles = ctx.enter_context(tc.tile_pool(name="singles", bufs=1))
            qkv = ctx.enter_context(tc.tile_pool(name="qkv", bufs=1))
            xpool = ctx.enter_context(tc.tile_pool(name="xpool", bufs=4))
            ptpool = ctx.enter_context(tc.tile_pool(name="ptpool", bufs=11))
            vpool = ctx.enter_context(tc.tile_pool(name="vpool", bufs=2))
            rpool = ctx.enter_context(tc.tile_pool(name="rpool", bufs=2))
            opool = ctx.enter_context(tc.tile_pool(name="opool", bufs=5))
            sppool = ctx.enter_context(
                tc.tile_pool(name="sppool", bufs=2, space="PSUM")
            )
            pbank = ctx.enter_context(tc.tile_pool(name="pbank", bufs=4, space="PSUM"))
            pools = (singles, qkv, xpool, ptpool, vpool, rpool, opool, sppool,
                     pbank, xt4, wqT, wkT, wvT, woT, bq, bk, bv, out)

            if dyn_reps:
                reps_sb = singles.tile([1, 1], mybir.dt.int32, name="reps_sb")
                nc.sync.dma_start(out=reps_sb[:], in_=reps[:])
                reps_val = nc.values_load(reps_sb[:], min_val=0, max_val=1 << 20)
                rep_loop = tc.For_i(0, reps_val, 1)
            elif n_reps > 1:
                rep_loop = tc.For_i(0, n_reps, 1)
            else:
                rep_loop = nullcontext()
            with rep_loop:
                _kernel_body(nc, tc, pools, phases)

    nc.finalize()
    return nc


def _emit_recips(nc, rpool, ctx_ps, bb, h):
    # DVE-only: reciprocal of the softmax sums, then a fp16 copy for the PE
    # broadcast matmul. Emitted eagerly at pair end.
    rs = rpool.tile([1, S], F32, tag="rs", name=f"rs_{bb}_{h}")
    for qs in range(NQS):
        nc.vector.reciprocal(
            rs[:, qs * TW : (qs + 1) * TW], ctx_ps[qs][HD : HD + 1, :]
        )
    rs_h = rpool.tile([1, S], F16, tag="rs_h", name=f"rs_h_{bb}_{h}")
    nc.vector.tensor_copy(rs_h[:], rs[:])
    return rs_h


def _emit_division(nc, rpool, ctx_ps, CX, pb, base, bb, h, rs_h):
    # GpSimd broadcasts the reciprocal row to 64 partitions (SBUF->SBUF,
    # GpSimd is otherwise idle) -- no PE outer product, no PSUM tile, so the
    # division never contends with the next pair's scores double-buffer.
    rb = rpool.tile([HD, S], F16, tag="rb", name=f"rb_{bb}_{h}")
    nc.gpsimd.partition_broadcast(rb[:, :], rs_h[:, :])
    for qs in range(NQS):
        nc.vector.tensor_mul(
            out=CX[pb : pb + HD, base + qs * TW : base + (qs + 1) * TW],
            in0=ctx_ps[qs][0:HD, :],
            in1=rb[:, qs * TW : (qs + 1) * TW],
        )


F32R = mybir.dt.float32r


def _microbench(nc, pools, phases):
    # Engine-rate microbenches (timing only, out never written).
    (singles, qkv, xpool, ptpool, vpool, rpool, opool, sppool, pbank,
     *_rest) = pools
    if phases.startswith("actbench"):
        sp = sppool.tile([P, HW2], F32, tag="sp", name="ab_src")
        nc.vector.memset(sp[:], 0.25)
        odt = F32 if phases == "actbench32" else F16
        for i in range(256):
            pt = ptpool.tile([P, HW2], odt, tag="pt", name=f"ab_{i}")
            nc.scalar.activation(pt[:], sp[:], AF.Exp, scale=float(SCALE))
        return
    # mmbench16 / mmbenchr: 512 score-like matmuls (K=64, N=512)
    lhs32 = singles.tile([HD, P], F32, tag="mb_lhs")
    rhs32 = singles.tile([HD, HW2], F32, tag="mb_rhs")
    nc.vector.memset(lhs32[:], 0.125)
    nc.vector.memset(rhs32[:], 0.125)
    if phases == "mmbench16":
        lhs = singles.tile([HD, P], F16, tag="mb_lhs16")
        rhs = singles.tile([HD, HW2], F16, tag="mb_rhs16")
        nc.vector.tensor_copy(lhs[:], lhs32[:])
        nc.vector.tensor_copy(rhs[:], rhs32[:])
    else:
        lhs = lhs32.bitcast(F32R)
        rhs = rhs32.bitcast(F32R)
    if phases == "mmbench128":
        # 512 N=128 matmuls: per-MM slope vs mmbench16 isolates PE clock
        for i in range(128):
            sp = sppool.tile([P, TW], F32, tag="sp", name=f"mb_{i}")
            for qs in range(4):
                nc.tensor.matmul(
                    sp[:, qs * 128 : (qs + 1) * 128],
                    lhs[:],
                    rhs[:, qs * 128 : (qs + 1) * 128],
                    start=True,
                    stop=True,
                )
        return
    if phases == "mmbenchst":
        # K=128 rotating stationary, all start=True: isolates the
        # start-clear cost from the K=64 effect
        wts = []
        for i in range(8):
            w32 = singles.tile([P, P], F32, tag=f"mb_w{i}")
            nc.vector.memset(w32[:], 0.03125)
            w16 = singles.tile([P, P], F16, tag=f"mb_w16{i}")
            nc.vector.tensor_copy(w16[:], w32[:])
            wts.append(w16)
        big32 = singles.tile([P, TW], F32, tag="mb_big")
        nc.vector.memset(big32[:], 0.0625)
        big = singles.tile([P, TW], F16, tag="mb_big16")
        nc.vector.tensor_copy(big[:], big32[:])
        for i in range(512):
            sp = sppool.tile([P, TW], F32, tag="sp", name=f"mbs_{i}")
            nc.tensor.matmul(
                sp[:], wts[i % 8][:], big[:], start=True, stop=True
            )
        return
    if phases == "mmbenchacc":
        # proj-like: 64 groups x 8 accumulating K=128 N=512 matmuls,
        # stationary rotates through 8 tiles
        wts = []
        for i in range(8):
            w32 = singles.tile([P, P], F32, tag=f"mb_w{i}")
            nc.vector.memset(w32[:], 0.03125)
            w16 = singles.tile([P, P], F16, tag=f"mb_w16{i}")
            nc.vector.tensor_copy(w16[:], w32[:])
            wts.append(w16)
        big32 = singles.tile([P, TW], F32, tag="mb_big")
        nc.vector.memset(big32[:], 0.0625)
        big = singles.tile([P, TW], F16, tag="mb_big16")
        nc.vector.tensor_copy(big[:], big32[:])
        for i in range(64):
            sp = sppool.tile([P, TW], F32, tag="sp", name=f"mba_{i}")
            for ko in range(8):
                nc.tensor.matmul(
                    sp[:],
                    wts[ko][:],
                    big[:],
                    start=(ko == 0),
                    stop=(ko == 7),
                )
        return
    for i in range(256):
        sp = sppool.tile([P, HW2], F32, tag="sp", name=f"mb_{i}")
        for qs in range(2):
            nc.tensor.matmul(
                sp[:, qs * TW : (qs + 1) * TW],
                lhs[:],
                rhs[:, qs * TW : (qs + 1) * TW],
                start=True,
                stop=True,
            )


def _kernel_body(nc, tc, pools, phases="full"):
    (singles, qkv, xpool, ptpool, vpool, rpool, opool, sppool, pbank,
     xt4, wqT, wkT, wvT, woT, bq, bk, bv, out) = pools

    if phases.startswith(("actbench", "mmbench")):
        _microbench(nc, pools, phases)
        return

    # ---- weights / biases to SBUF ----
    w_sbs = []
    for name, wT in (("wq", wqT), ("wk", wkT), ("wv", wvT)):
        w_sb = singles.tile([P, KO, P], F16, tag=f"{name}_sb")
        nc.sync.dma_start(out=w_sb[:], in_=wT[:])
        w_sbs.append(w_sb)
    wo_sb = singles.tile([P, D], F16, tag="wo_sb")
    nc.sync.dma_start(out=wo_sb[:], in_=woT[:])
    b_sbs = []
    for name, bdram in (("bq", bq), ("bk", bk), ("bv", bv)):
        b_sb = singles.tile([P, 1], F32, tag=f"{name}_sb")
        nc.sync.dma_start(out=b_sb[:], in_=bdram[:])
        b_sbs.append(b_sb)

    # ones row for PE-broadcast of softmax reciprocals (K=1 matmul)
    negone = singles.tile([P, 1], F32, tag="negone")
    nc.vector.memset(negone[:], -2.5)

    # ---- persistent activations ----
    QT = qkv.tile([P, T], F16, tag="QT")
    # K is kept zero-padded to K=128 per head (ZK0 = [K_h0; 0], ZK1 =
    # [0; K_h1]) so every scores matmul has a full 128-row stationary:
    # K=64 stationaries disqualify fast-weight-load and cost ~190ns/MM extra
    # (HW-measured). The zero half multiplies the other head's Q rows.
    ZK0 = qkv.tile([P, T], F16, tag="ZK0")
    ZK1 = qkv.tile([P, T], F16, tag="ZK1")
    VT = qkv.tile([P, T], F16, tag="VT")
    CX = qkv.tile([P, T], F16, tag="CX")  # scaled ctxT, both heads
    nc.gpsimd.memset(ZK0[HD:P, :], 0.0)
    nc.gpsimd.memset(ZK1[0:HD, :], 0.0)

    # ---- projections: QT/KT/VT[f, t] = sum_d W[d, f] * xT[d, t] ----

    xts = {}

    def proj_tile(tt, parts="QKV"):
        if tt in xts:
            xt = xts[tt]
        else:
            xt = xpool.tile([P, KO, TW], F16, tag="xt", name=f"xt_{tt}")
            # four dma_starts -> four HWDGE queues in parallel (8-way split
            # measured same-window neutral-to-worse)
            nq = 4
            w = KO // nq
            for q in range(nq):
                nc.sync.dma_start(
                    out=xt[:, w * q : w * (q + 1)],
                    in_=xt4[:][tt, :, w * q : w * (q + 1)],
                )
            xts[tt] = xt
        for which, w_sb, b_sb, dst in zip(
            "QKV", w_sbs, b_sbs, (QT, None, VT), strict=True
        ):
            if which not in parts:
                continue
            ps = sppool.tile([P, TW], F32, tag="sp")
            for ko in range(KO):
                nc.tensor.matmul(
                    ps[:],
                    w_sb[:, ko],
                    xt[:, ko],
                    start=(ko == 0),
                    stop=(ko == KO - 1),
                )
                if ko == KO // 2 - 1:
                    # mid-unit yield: interleaved insertions stay ~1.1us so
                    # they don't starve the exp cadence (PE-bound on HW)
                    yield
            sl = slice(tt * TW, (tt + 1) * TW)
            if dst is None:  # K projection: split into the zero-padded bufs
                nc.vector.tensor_scalar_add(
                    ZK0[0:HD, sl], ps[0:HD, :], b_sb[0:HD]
                )
                nc.vector.tensor_scalar_add(
                    ZK1[HD:P, sl], ps[HD:P, :], b_sb[HD:P]
                )
            else:
                nc.vector.tensor_scalar_add(dst[:, sl], ps[:], b_sb[:])
            yield

    # batch-0 Q and K projections first; the V projections and the V' build
    # are deferred into pair(0,0)'s early grains (V' is first consumed at the
    # first ctx matmul, `lag` grains in)
    for tt in range(NT // 2):
        for _ in proj_tile(tt, "QK"):
            pass

    # ---- V' build for one (batch, head): V plus a ones column, via DVE
    #      32x32 stream transposes of VT (fp16 end to end) ----
    def vbuild_into(vp, bb: int, h: int):
        base = bb * S
        pb = h * HD
        nc.vector.memset(vp[:, :, HD], 1.0)
        for a in range(2):
            src = VT[pb + 32 * a : pb + 32 * (a + 1), base : base + S]
            src = src.rearrange("p (kc r) -> p kc r", r=P)
            for b2 in range(4):
                nc.vector.transpose(
                    vp[32 * b2 : 32 * (b2 + 1), :, 32 * a : 32 * (a + 1)],
                    src[:, :, 32 * b2 : 32 * (b2 + 1)],
                )
        if not USE_FP8_CTX:
            return vp
        # fp8 copy, inner dim padded to 80 so the DoubleRow Ko stride is
        # 16-byte aligned
        vp8 = vpool.tile([P, NKC, 80], F8, tag="vp8", name=f"vp8_{bb}_{h}")
        nc.vector.tensor_copy(vp8[:, :, 0 : HD + 1], vp[:])
        return vp8

    def vbuild(bb: int, h: int):
        vp = vpool.tile([P, NKC, HD + 1], F16, tag="vp", name=f"vp_{bb}_{h}")
        return vbuild_into(vp, bb, h)

    # ---- attention for one (batch, head); generator yields per grain
    #      (k-chunk x q-half), ctx matmuls lag one grain behind scores/exp
    #      so the PE FIFO never stalls head-of-queue on the exp handoff ----
    def pair_attn(bb: int, h: int, vp, lag: int = 1, drain_early: bool = False):
        base = bb * S       # token offset of this batch
        pb = h * HD         # partition offset of this head in QT/KT/VT
        yield

        ctx_ps = [
            pbank.tile([HD + 1, TW], F32, tag="pb", name=f"ctx_ps_{bb}_{h}_{i}")
            for i in range(NQS)
        ]

        ZK = ZK1 if h else ZK0

        def emit_scores(kc, half, sp):
            kt_chunk = ZK[:, base + kc * P : base + (kc + 1) * P]
            q0 = half * HW2
            for qs in range(2):
                nc.tensor.matmul(
                    sp[:, qs * TW : (qs + 1) * TW],
                    kt_chunk,
                    QT[
                        :,
                        base + q0 + qs * TW : base + q0 + (qs + 1) * TW,
                    ],
                    start=True,
                    stop=True,
                )

        from collections import deque

        if USE_FP8_CTX:
            NPR = NKC // 2

            def emit_se8(c, j, half, pp):
                sp = sppool.tile(
                    [P, HW2], F32, tag="sp", name=f"sp_{bb}_{h}_{c}_{j}_{half}"
                )
                emit_scores(2 * c + j, half, sp)
                nc.scalar.activation(
                    pp[:, j], sp[:], AF.Exp, scale=float(SCALE), bias=negone[:]
                )

            def emit_ctx8(c, half, pp):
                vpair = vp[:, 2 * c : 2 * c + 2, 0 : HD + 1]
                for qs in range(2):
                    nc.tensor.matmul(
                        ctx_ps[half * 2 + qs][:],
                        vpair,
                        pp[:, :, qs * TW : (qs + 1) * TW],
                        start=(c == 0),
                        stop=(c == NPR - 1),
                        perf_mode=mybir.MatmulPerfMode.DoubleRow,
                    )

            pend = deque()
            pps = {}
            for c in range(NPR):
                for j in range(2):
                    for half in range(2):
                        if j == 0:
                            pps[half] = ptpool.tile(
                                [P, 2, HW2], F8, tag="pt",
                                name=f"pp_{bb}_{h}_{c}_{half}",
                            )
                        emit_se8(c, j, half, pps[half])
                        if j == 1:
                            pend.append((c, half, pps[half]))
                        if len(pend) >= 2:
                            emit_ctx8(*pend.popleft())
                        yield
            while pend:
                emit_ctx8(*pend.popleft())
        else:

            def emit_se(kc, half):
                sp = sppool.tile(
                    [P, HW2], F32, tag="sp", name=f"sp_{bb}_{h}_{kc}_{half}"
                )
                emit_scores(kc, half, sp)
                pt = ptpool.tile(
                    [P, HW2], F16, tag="pt", name=f"pt_{bb}_{h}_{kc}_{half}"
                )
                nc.scalar.activation(pt[:], sp[:], AF.Exp, scale=float(SCALE))
                return pt

            def emit_ctx(kc, half, pt):
                vchunk = vp[:, kc]
                for qs in range(2):
                    nc.tensor.matmul(
                        ctx_ps[half * 2 + qs][:],
                        vchunk,
                        pt[:, qs * TW : (qs + 1) * TW],
                        start=(kc == 0),
                        stop=(kc == NKC - 1),
                    )

            pending = deque()
            gi = 0
            for kc in range(NKC):
                for half in range(2):
                    pt = emit_se(kc, half)
                    gi += 1
                    # drain_early: work the backlog down mid-pair (2 ctx/grain
                    # once past grain 20) so the final division isn't gated by
                    # a lag-deep ctx burst at the very end
                    limit = 2 if (drain_early and gi > 20) else lag
                    pops = 0
                    while len(pending) >= limit and pops < 2:
                        emit_ctx(*pending.popleft())
                        pops += 1
                    pending.append((kc, half, pt))
                    yield
            while pending:
                emit_ctx(*pending.popleft())

        rs_h = _emit_recips(nc, rpool, ctx_ps, bb, h)

        def division():
            _emit_division(nc, rpool, ctx_ps, CX, pb, base, bb, h, rs_h)

        yield division

    # ---- output projection for one batch (generator) ----
    def outproj(bb: int, tail: bool = False):
        for tci in range(S // P):
            tg = bb * (S // P) + tci
            ot = opool.tile([P, D], F16, tag="ot")
            for half in range(2):
                ps = sppool.tile([P, TW], F32, tag="sp")
                nc.tensor.matmul(
                    ps[:],
                    CX[:, tg * P : (tg + 1) * P],
                    wo_sb[:, half * TW : (half + 1) * TW],
                    start=True,
                    stop=True,
                )
                # DVE evacuation while interleaved with attention (ScalarE is
                # saturated by exp there); at the tail ScalarE is idle, so
                # split the halves across both engines
                if tail and half == 0:
                    nc.scalar.copy(ot[:, half * TW : (half + 1) * TW], ps[:])
                else:
                    nc.vector.tensor_copy(
                        ot[:, half * TW : (half + 1) * TW], ps[:]
                    )
                nc.sync.dma_start(
                    out=out[:][
                        tg * P : (tg + 1) * P, half * TW : (half + 1) * TW
                    ],
                    in_=ot[:, half * TW : (half + 1) * TW],
                )
            yield

    if phases == "proj":
        for tt in range(NT // 2, NT):
            for _ in proj_tile(tt):
                pass
        for i, t_ in enumerate((QT, ZK0, VT)):
            for j in range(4):
                nc.sync.dma_start(
                    out=out[:][(4 * i + j) * P : (4 * i + j + 1) * P, :],
                    in_=t_[:, j * D : (j + 1) * D],
                )
        return

    def run_pair(gen, prev_div=None, interleave=None, plan=None):
        # Drive a pair generator. Yields are: one pre-yield, one per grain,
        # then the pair's deferred division closure. The previous pair's
        # division is emitted right after this pair's first grain; `plan`
        # maps grain number -> how many interleave steps to emit there.
        division = None
        n = 0
        for item in gen:
            if callable(item):
                division = item
                continue
            n += 1
            if prev_div is not None and n == 1:
                prev_div()
                prev_div = None
            if interleave is not None and plan:
                for _ in range(plan.get(n, 0)):
                    next(interleave, None)
        if prev_div is not None:
            prev_div()
        return division

    def spread(grains, count, start=1, step=2):
        # plan placing `count` steps at grains start, start+step, ...
        return {start + i * step: 1 for i in range(count)}

    if phases == "oldsched":
        # previous schedule (serial b0 V-proj + vbuild before pair(0,0)),
        # kept for same-window A/B against the deferred-V flow
        for tt in range(NT // 2):
            for _ in proj_tile(tt, "V"):
                pass
        proj_steps = itertools.chain(
            *[proj_tile(tt) for tt in range(NT // 2, NT)]
        )
        vp0 = vbuild(0, 0)
        d00 = run_pair(
            pair_attn(0, 0, vp0), None, proj_steps, plan=spread(32, 12, 2, 3)
        )
        for _ in proj_steps:
            pass
        vp_ = vbuild(0, 1)
        d01 = run_pair(pair_attn(0, 1, vp_), prev_div=d00)
        opg = outproj(0)
        vp_ = vbuild(1, 0)
        d10 = run_pair(
            pair_attn(1, 0, vp_), prev_div=d01, interleave=opg,
            plan=spread(32, 14, 5, 2),
        )
        vp_ = vbuild(1, 1)
        d11 = run_pair(pair_attn(1, 1, vp_), prev_div=d10)
        d11()
        for _ in opg:
            pass
        for _ in outproj(1, tail=True):
            pass
        return

    if phases in ("attn1", "attn1i"):
        # diagnostic: one attention pair, with vs without proj interleave
        # (same total emitted work either way)
        for tt in range(NT // 2):
            for _ in proj_tile(tt, "V"):
                pass
        if phases == "attn1":
            for tt in range(NT // 2, NT):
                for _ in proj_tile(tt):
                    pass
        vp0 = vbuild(0, 0)
        if phases == "attn1":
            d = run_pair(pair_attn(0, 0, vp0))
        else:
            steps = itertools.chain(
                *[proj_tile(tt) for tt in range(NT // 2, NT)]
            )
            d = run_pair(
                pair_attn(0, 0, vp0), None, steps, plan=spread(32, 12, 2, 3)
            )
            for _ in steps:
                pass
        d()
        nc.sync.dma_start(out=out[:][0:P, :], in_=CX[:, 0:D])
        return

    # pair(0,0) starts right after the batch-0 Q/K projections. Its early
    # grains (every grain, 1..5) emit the deferred batch-0 V projections and
    # the V' build -- all done by grain 5, safely before the first ctx
    # matmul at grain lag+1 = 9. The batch-1 projections are then spread
    # over the rest of pair(0,0) and pair(0,1); outproj(0) over pairs (1,0)
    # and (1,1). Each pair's division is deferred into the next pair's first
    # grain (prev_div) so the recip->broadcast->mul chain never stalls the
    # PE FIFO at a pair boundary.
    vp00 = vpool.tile([P, NKC, HD + 1], F16, tag="vp", name="vp_0_0")

    def late_steps():
        for tt in range(NT // 2):
            for _ in proj_tile(tt, "V"):
                yield
        vbuild_into(vp00, 0, 0)
        yield
        for tt in range(NT // 2, NT):
            for _ in proj_tile(tt):
                yield

    def one_shot(fn_):
        fn_()
        yield

    steps = late_steps()  # 8 V-proj halves + 1 vbuild + 24 batch-1 halves
    plan00 = {n: 1 for n in range(1, 10)}
    plan00.update(spread(32, 12, 10, 2))  # grains 10,12,...,32
    div00 = run_pair(
        pair_attn(0, 0, vp00, lag=10, drain_early=phases != "de1"), None,
        steps, plan=plan00,
    )

    # later pairs: V' build lands at grain 2 (after the previous division's
    # DVE chain releases the ctx banks), first ctx at grain lag+1 = 9
    vp01 = vpool.tile([P, NKC, HD + 1], F16, tag="vp", name="vp_0_1")
    steps01 = itertools.chain(
        one_shot(lambda: vbuild_into(vp01, 0, 1)), steps
    )
    div01 = run_pair(
        pair_attn(0, 1, vp01, lag=8, drain_early=phases != "de1"),
        prev_div=div00, interleave=steps01,
        plan={2: 1, **spread(32, 12, 3, 2)},
    )
    for _ in steps01:
        pass
    if phases == "attn2":
        div01()
        nc.sync.dma_start(out=out[:][0:P, :], in_=CX[:, 0:D])
        return
    op0 = outproj(0)
    vp10 = vpool.tile([P, NKC, HD + 1], F16, tag="vp", name="vp_1_0")
    steps10 = itertools.chain(
        one_shot(lambda: vbuild_into(vp10, 1, 0)), op0
    )
    div10 = run_pair(
        pair_attn(1, 0, vp10, lag=8, drain_early=phases != "de1"), prev_div=div01, interleave=steps10,
        plan={2: 1, **spread(32, 14, 4, 2)},
    )
    vp11 = vpool.tile([P, NKC, HD + 1], F16, tag="vp", name="vp_1_1")
    steps11 = itertools.chain(
        one_shot(lambda: vbuild_into(vp11, 1, 1)), steps10
    )
    div11 = run_pair(
        pair_attn(1, 1, vp11, lag=8, drain_early=True), prev_div=div10,
        interleave=steps11, plan={2: 1, 4: 1, 6: 1},
    )
    div11()
    for _ in steps11:
        pass
    for _ in outproj(1, tail=True):
        pass


@functools.lru_cache(maxsize=8)
def _get_nc(n_reps: int = 1, phases: str = "full", dyn_reps: bool = False):
    return _build_nc(n_reps, phases, dyn_reps)


def _shard_inputs(x, Wq, bq, Wk, bk, Wv, bv, Wo, bo):
    x = np.asarray(x, dtype=np.float32)
    # xt4[tt, ki, ko, t] = x[tt*TW + t, ko*P + ki]
    xt4 = np.ascontiguousarray(
        x.reshape(NT, TW, KO, P).transpose(0, 3, 2, 1)
    ).astype(np.float16)
    Wq = np.asarray(Wq, dtype=np.float32)
    Wk = np.asarray(Wk, dtype=np.float32)
    Wv = np.asarray(Wv, dtype=np.float32)
    Wo = np.asarray(Wo, dtype=np.float32)
    bq = np.asarray(bq, dtype=np.float32)
    bk = np.asarray(bk, dtype=np.float32)
    bv = np.asarray(bv, dtype=np.float32)

    def wtile(W, sl):
        # [ki, ko, f] = W[c*P + f, ko*P + ki]
        return np.ascontiguousarray(
            W[sl, :].reshape(P, KO, P).transpose(2, 1, 0)
        ).astype(np.float16)

    in_maps = []
    for c in range(N_CORES):
        sl = slice(c * P, (c + 1) * P)
        in_maps.append(
            {
                "xt4": xt4,
                "wqT": wtile(Wq, sl),
                "wkT": wtile(Wk, sl),
                "wvT": wtile(Wv, sl),
                "woT": np.ascontiguousarray(Wo[:, sl].T).astype(np.float16),
                "bq": np.ascontiguousarray(bq[sl].reshape(P, 1)),
                "bk": np.ascontiguousarray(bk[sl].reshape(P, 1)),
                "bv": np.ascontiguousarray(bv[sl].reshape(P, 1)),
            }
        )
    return in_maps


def kernel(x, Wq, bq, Wk, bk, Wv, bv, Wo, bo, **run_kwargs):
    nc = _get_nc()
    in_maps = _shard_inputs(x, Wq, bq, Wk, bk, Wv, bv, Wo, bo)
    last_exc = None
    for _attempt in range(3):
        try:
            res = run_bass_kernel_spmd(
                nc, in_maps, core_ids=list(range(N_CORES)), **run_kwargs
            )
            break
        except Exception as exc:  # transient device errors: retry
            last_exc = exc
            import time as _time

            _time.sleep(3.0)
            # a wedged PJRT client never recovers in-process; force a fresh
            # backend connection so the retry sees recovered devices
            try:
                import jax as _jax

                _jax.clear_caches()
                _jax.extend.backend.clear_backends()
            except Exception:
                pass
    else:
        raise last_exc
    partials = [r["out"] for r in res.results]
    acc = np.add.reduce([np.asarray(p, dtype=np.float32) for p in partials], axis=0)
    acc = acc + np.asarray(bo, dtype=np.float32)[None, :]
    if run_kwargs:
        kernel.last_results = res
    return acc.reshape(B, S, D).astype(np.float32)
